# revision 8
# baseline (speedup 1.0000x reference)
"""Trainium2 kernel for nn_DeepPatchEncoder.

The reference pipeline (patchify16 + pos_emb -> unpatchify -> patchify8 +
pos_new -> unpatchify -> patchify16) collapses algebraically: patchify /
unpatchify are inverse permutations, so

    out = patchify16(X + Z),   Z = unpatchify16(pos_emb) + unpatchify8(pos_new)

where Z is a single [224,224,3] image computed from the tiny parameters
(pos_emb conv + batchnorm).  Z is computed on host in numpy (O(100KB) of
work); the per-sample memory-bound add + patch permutation runs on 8
NeuronCores, data-parallel over the batch (16 samples per core).

Per core the work is laid out as 224 independent blocks (sample b x coarse
row i).  Block input = 16 consecutive image rows (10752 floats, contiguous
in DRAM); block output = 14 consecutive encoder rows (10752 floats,
contiguous in DRAM).  Within a block the map is a pure (p0:16 <-> j:14)
axis swap of 48-float chunks, done on the VectorEngine as a single
tensor_tensor add with strided access patterns (which also adds Z).
All DMAs stay linear / large-chunk.
"""
import sys

for _p in ("/opt/trn_rl_repo", "/root/.axon_site/_ro/trn_rl_repo",
           "/root/.axon_site/_ro/pypackages"):
    if _p not in sys.path:
        sys.path.append(_p)

import numpy as np
import concourse.bass as bass
import concourse.bacc as bacc
import concourse.mybir as mybir
import concourse.tile as tile
from concourse.bass_utils import run_bass_kernel_spmd

F32 = mybir.dt.float32

B, IMG, C = 128, 224, 3
P0, P1 = 16, 8
N0 = (IMG // P0) ** 2   # 196
D0 = C * P0 * P0        # 768
BN_EPS = 1e-3

NCORES = 8
NB = B // NCORES        # 16 samples per core
NI = IMG // P0          # 14 coarse rows
NBLK = NB * NI          # 224 blocks per core
ROWF = IMG * C          # 672 floats per image row
FREE = P0 * ROWF        # 10752 floats per block
P = 112                 # partitions per tile
NT = NBLK // P          # 2 tiles
NH = 2                  # split free dim in j-halves for SBUF fit
JH = NI // NH           # 7
HFREE = FREE // NH      # 5376


def _compute_z(pos_emb, conv_w, bn_gamma, bn_beta, bn_mean, bn_var):
    """The [224,224,3] constant image Z (all-numpy, host side)."""
    pos_emb = np.asarray(pos_emb, np.float32)
    # unpatchify16(pos_emb): [196,768] -> [224,224,3]
    q = pos_emb.reshape(14, 14, P0, P0, C).transpose(0, 2, 1, 3, 4)
    q = q.reshape(IMG, IMG, C)

    # pos pipeline: [3,16,16,196] -conv2x2s2-> [3,8,8,784] -> BN
    pos_img = pos_emb.reshape(N0, P0, P0, C).transpose(3, 1, 2, 0)
    v = pos_img.reshape(C, 8, 2, 8, 2, N0).astype(np.float64)
    pos_c = np.einsum("nidjec,deco->nijo", v, np.asarray(conv_w, np.float64))
    inv = np.asarray(bn_gamma, np.float64) / np.sqrt(
        np.asarray(bn_var, np.float64) + BN_EPS)
    pos_c = (pos_c - np.asarray(bn_mean, np.float64)) * inv + np.asarray(
        bn_beta, np.float64)
    pos_new = pos_c.transpose(3, 1, 2, 0).astype(np.float32)  # [784,8,8,3]

    # unpatchify8(pos_new): [784,8,8,3] -> [224,224,3]
    r = pos_new.reshape(28, 28, P1, P1, C).transpose(0, 2, 1, 3, 4)
    r = r.reshape(IMG, IMG, C)
    return q + r


_NC_CACHE = None


def _build_kernel():
    global _NC_CACHE
    if _NC_CACHE is not None:
        return _NC_CACHE
    nc = bacc.Bacc()
    x = nc.declare_dram_parameter("x", [NBLK, FREE], F32, isOutput=False)
    z = nc.declare_dram_parameter("z", [NI, FREE], F32, isOutput=False)
    out = nc.declare_dram_parameter("out", [NBLK, FREE], F32, isOutput=True)

    with tile.TileContext(nc) as tc:
        with (
            tc.tile_pool(name="zpool", bufs=1) as zpool,
            tc.tile_pool(name="xp", bufs=3) as xp,
            tc.tile_pool(name="op", bufs=2) as op,
        ):
            # Z replicated across all 112 partitions (i = partition % 14):
            # one HBM load of the 14 distinct rows, then SBUF->SBUF doubling.
            zrep = zpool.tile([P, FREE], F32, tag="zrep")
            nc.sync.dma_start(out=zrep[0:14, :], in_=z[:, :])
            nc.sync.dma_start(out=zrep[14:28, :], in_=zrep[0:14, :])
            nc.sync.dma_start(out=zrep[28:56, :], in_=zrep[0:28, :])
            nc.sync.dma_start(out=zrep[56:112, :], in_=zrep[0:56, :])
            # Wait-funnel: cheap DVE reads of every zrep partition absorb
            # the 4 DMA-lane waits so the per-tile TensorTensors (same
            # engine, program-ordered after this) don't each re-wait on
            # them -- ISA instruction structs have few sync-wait slots,
            # so split the funnel so each op carries at most 2 waits.
            zscratch = zpool.tile([P, 2], F32, tag="zscratch")
            nc.vector.tensor_copy(zscratch[0:28, :], zrep[0:28, 0:2])
            # reading any partition a DMA wrote is enough for the engine to
            # observe that DMA's sem tick; [32:64) touches both the [28:56)
            # and [56:112) writers (non-zero base caps the span at 32)
            nc.vector.tensor_copy(zscratch[32:64, :], zrep[32:64, 0:2])
            # per-partition layout (p0:16, j:14, k:48) viewed as (j, p0, k)
            zv = zrep[:].rearrange("p (p0 j k) -> p j p0 k", p0=P0, j=NI, k=48)

            for t in range(NT):
                rows = x[t * P:(t + 1) * P, :]
                rowsv = rows.rearrange("r (p0 h m) -> r p0 h m",
                                       p0=P0, h=NH, m=HFREE // P0)
                for h in range(NH):
                    xt = xp.tile([P, HFREE], F32, tag="xt")
                    ot = op.tile([P, HFREE], F32, tag="ot")
                    nc.sync.dma_start(out=xt[:], in_=rowsv[:, :, h, :])
                    in0 = xt[:].rearrange("p (p0 j k) -> p j p0 k",
                                          p0=P0, j=JH, k=48)
                    in1 = zv[:, h * JH:(h + 1) * JH, :, :]
                    o0 = ot[:].rearrange("p (j p0 k) -> p j p0 k",
                                         j=JH, p0=P0, k=48)
                    nc.vector.tensor_tensor(o0, in0, in1, mybir.AluOpType.add)
                    nc.sync.dma_start(
                        out=out[t * P:(t + 1) * P, h * HFREE:(h + 1) * HFREE],
                        in_=ot[:])
    nc.finalize()  # runs Bacc.compile(): reg alloc + excess-wait splitting
    _NC_CACHE = nc
    return nc


def kernel(X, pos_emb, conv_w, bn_gamma, bn_beta, bn_mean, bn_var,
           _spmd_kwargs=None):
    X = np.ascontiguousarray(np.asarray(X, np.float32))
    zimg = _compute_z(pos_emb, conv_w, bn_gamma, bn_beta, bn_mean, bn_var)
    z_np = np.ascontiguousarray(zimg.reshape(NI, FREE))

    nc = _build_kernel()
    in_maps = []
    for c in range(NCORES):
        shard = X[c * NB:(c + 1) * NB].reshape(NBLK, FREE)
        in_maps.append({"x": np.ascontiguousarray(shard), "z": z_np})

    res = run_bass_kernel_spmd(nc, in_maps, list(range(NCORES)),
                               **(_spmd_kwargs or {}))

    out = np.empty((B, N0, D0), np.float32)
    for c in range(NCORES):
        out[c * NB:(c + 1) * NB] = res.results[c]["out"].reshape(NB, N0, D0)
    if _spmd_kwargs:
        kernel.last_results = res
    return out


# revision 9
# speedup vs baseline: 1.3721x; 1.3721x over previous
"""Trainium2 kernel for nn_DeepPatchEncoder.

The reference pipeline (patchify16 + pos_emb -> unpatchify -> patchify8 +
pos_new -> unpatchify -> patchify16) collapses algebraically: patchify /
unpatchify are inverse permutations, so

    out = patchify16(X + Z),   Z = unpatchify16(pos_emb) + unpatchify8(pos_new)

where Z is a single [224,224,3] image computed from the tiny parameters
(pos_emb conv + batchnorm).  Z is computed on host in numpy (O(100KB) of
work); the per-sample memory-bound add + patch permutation runs on 8
NeuronCores, data-parallel over the batch (16 samples per core).

Per core the work is 224 independent blocks (sample b x coarse row i).
Block input = 16 consecutive image rows (10752 floats, contiguous in
DRAM); block output = 14 consecutive encoder rows (10752 floats,
contiguous in DRAM).  Within a block the map is a pure (p0:16 <-> j:14)
axis swap of 48-float chunks, done on the VectorEngine as tensor_tensor
adds with strided access patterns (which also add Z).  All DMAs are
large contiguous transfers; loads ride the SP HWDGE ring and stores the
ACT ring.  Z is replicated across the 112 partitions (zrep[p] = z[p%14])
on the otherwise-idle TensorEngine via a one-hot selection matmul, with
ScalarEngine PSUM->SBUF copies -- keeping the replication off both the
DMA engines and the VectorEngine.
"""
import sys

for _p in ("/opt/trn_rl_repo", "/root/.axon_site/_ro/trn_rl_repo",
           "/root/.axon_site/_ro/pypackages"):
    if _p not in sys.path:
        sys.path.append(_p)

import numpy as np
import concourse.bass as bass
import concourse.bacc as bacc
import concourse.mybir as mybir
import concourse.tile as tile
from concourse.bass_utils import run_bass_kernel_spmd

F32 = mybir.dt.float32

B, IMG, C = 128, 224, 3
P0, P1 = 16, 8
N0 = (IMG // P0) ** 2   # 196
D0 = C * P0 * P0        # 768
BN_EPS = 1e-3

NCORES = 8
NB = B // NCORES        # 16 samples per core
NI = IMG // P0          # 14 coarse rows
NBLK = NB * NI          # 224 blocks per core
ROWF = IMG * C          # 672 floats per image row
FREE = P0 * ROWF        # 10752 floats per block
P = 112                 # partitions per tile
NT = NBLK // P          # 2 tiles
NH = 2                  # j-halves per tile (store granularity)
JH = NI // NH           # 7
HFREE = FREE // NH      # 5376
ZQ = 4                  # z chunks for the replication matmuls
ZQF = FREE // ZQ        # 2688
MMN = 512               # matmul moving-dim tile


def _compute_z(pos_emb, conv_w, bn_gamma, bn_beta, bn_mean, bn_var):
    """The [224,224,3] constant image Z (all-numpy, host side)."""
    pos_emb = np.asarray(pos_emb, np.float32)
    # unpatchify16(pos_emb): [196,768] -> [224,224,3]
    q = pos_emb.reshape(14, 14, P0, P0, C).transpose(0, 2, 1, 3, 4)
    q = q.reshape(IMG, IMG, C)

    # pos pipeline: [3,16,16,196] -conv2x2s2-> [3,8,8,784] -> BN
    pos_img = pos_emb.reshape(N0, P0, P0, C).transpose(3, 1, 2, 0)
    v = pos_img.reshape(C, 8, 2, 8, 2, N0).astype(np.float64)
    pos_c = np.einsum("nidjec,deco->nijo", v, np.asarray(conv_w, np.float64))
    inv = np.asarray(bn_gamma, np.float64) / np.sqrt(
        np.asarray(bn_var, np.float64) + BN_EPS)
    pos_c = (pos_c - np.asarray(bn_mean, np.float64)) * inv + np.asarray(
        bn_beta, np.float64)
    pos_new = pos_c.transpose(3, 1, 2, 0).astype(np.float32)  # [784,8,8,3]

    # unpatchify8(pos_new): [784,8,8,3] -> [224,224,3]
    r = pos_new.reshape(28, 28, P1, P1, C).transpose(0, 2, 1, 3, 4)
    r = r.reshape(IMG, IMG, C)
    return q + r


_NC_CACHE = None


def _build_kernel():
    global _NC_CACHE
    if _NC_CACHE is not None:
        return _NC_CACHE
    nc = bacc.Bacc()
    x = nc.declare_dram_parameter("x", [NBLK, FREE], F32, isOutput=False)
    z = nc.declare_dram_parameter("z", [NI, FREE], F32, isOutput=False)
    s = nc.declare_dram_parameter("s", [NI, P], F32, isOutput=False)
    out = nc.declare_dram_parameter("out", [NBLK, FREE], F32, isOutput=True)

    with tile.TileContext(nc) as tc:
        with (
            tc.tile_pool(name="cpool", bufs=1) as cpool,
            tc.tile_pool(name="zck", bufs=2) as zck,
            tc.tile_pool(name="zp", bufs=1) as zp,
            tc.tile_pool(name="ps", bufs=4, space="PSUM") as ps,
            tc.tile_pool(name="xp", bufs=2) as xp,
            tc.tile_pool(name="op", bufs=2) as op,
        ):
            s_tile = cpool.tile([NI, P], F32)
            nc.sync.dma_start(out=s_tile[:], in_=s[:, :])
            zrep = zp.tile([P, FREE], F32)
            # replicate z across partitions (zrep[p] = z[p % 14]) on the
            # idle TensorEngine: out[112, n] = S.T @ z_chunk, S one-hot
            for q in range(ZQ):
                zc = zck.tile([NI, ZQF], F32, tag="zc")
                nc.sync.dma_start(out=zc[:], in_=z[:, q * ZQF:(q + 1) * ZQF])
                for c0 in range(0, ZQF, MMN):
                    n = min(MMN, ZQF - c0)
                    pz = ps.tile([P, MMN], F32, tag="pz")
                    nc.tensor.matmul(pz[:, :n], s_tile[:], zc[:, c0:c0 + n],
                                     start=True, stop=True)
                    nc.scalar.copy(
                        out=zrep[:, q * ZQF + c0: q * ZQF + c0 + n],
                        in_=pz[:, :n])
            # per-partition z layout (p0:16, j:14, k:48) viewed as (j, p0, k)
            zv = zrep[:].rearrange("p (p0 j k) -> p j p0 k", p0=P0, j=NI, k=48)

            for t in range(NT):
                xt = xp.tile([P, FREE], F32, tag="xt")
                nc.sync.dma_start(out=xt[:], in_=x[t * P:(t + 1) * P, :])
                for h in range(NH):
                    ot = op.tile([P, HFREE], F32, tag="ot")
                    in0 = xt[:].rearrange(
                        "p (p0 j k) -> p j p0 k",
                        p0=P0, j=NI, k=48)[:, h * JH:(h + 1) * JH]
                    in1 = zv[:, h * JH:(h + 1) * JH]
                    o0 = ot[:].rearrange("p (j p0 k) -> p j p0 k",
                                         j=JH, p0=P0, k=48)
                    nc.vector.tensor_tensor(o0, in0, in1, mybir.AluOpType.add)
                    nc.scalar.dma_start(
                        out=out[t * P:(t + 1) * P, h * HFREE:(h + 1) * HFREE],
                        in_=ot[:])
    nc.finalize()
    _NC_CACHE = nc
    return nc


_S_NP = np.zeros((NI, P), np.float32)
for _pp in range(P):
    _S_NP[_pp % NI, _pp] = 1.0


def kernel(X, pos_emb, conv_w, bn_gamma, bn_beta, bn_mean, bn_var,
           _spmd_kwargs=None):
    X = np.ascontiguousarray(np.asarray(X, np.float32))
    zimg = _compute_z(pos_emb, conv_w, bn_gamma, bn_beta, bn_mean, bn_var)
    z_np = np.ascontiguousarray(zimg.reshape(NI, FREE))

    nc = _build_kernel()
    in_maps = []
    for c in range(NCORES):
        shard = X[c * NB:(c + 1) * NB].reshape(NBLK, FREE)
        in_maps.append({"x": np.ascontiguousarray(shard), "z": z_np,
                        "s": _S_NP})

    res = run_bass_kernel_spmd(nc, in_maps, list(range(NCORES)),
                               **(_spmd_kwargs or {}))

    out = np.empty((B, N0, D0), np.float32)
    for c in range(NCORES):
        out[c * NB:(c + 1) * NB] = res.results[c]["out"].reshape(NB, N0, D0)
    if _spmd_kwargs:
        kernel.last_results = res
    return out
